# revision 11
# baseline (speedup 1.0000x reference)
"""BPR matrix-factorization loss on 8 Trainium2 NeuronCores.

Strategy (data parallel): the host concatenates U [1M,64] and V [500K,64]
into one UV table [1.5M,64] replicated to every core's HBM, and offsets the
i/j item indices by NUM_USER so u/i/j all index one table. The batch of
16384 triples is sharded 2048 per core, laid out [128 partitions x 16
blocks]. Each core gathers all 6144 embedding rows (256 B each) with ONE
indirect DMA, computes row-wise dots on DVE and transcendental partial sums
on ACT, and writes a single [128, 38] result tile. The host concatenates
the y shards and finishes the scalar loss in float64 using
  sum(log_sigmoid(x)) = 0.5*sum(x) - 0.5*sum(|x|) - sum(ln(1+exp(-|x|))).

TRN2 allows one sync-wait per compute instruction and few per drain, so the
kernel touches only 5 semaphores (idx DMA, gather, out DMA, DVE, ACT): a 1x1
DVE "touch" absorbs the gather semaphore, scratch/accum tiles are never
shared between ops, and no compute op mixes ACT- and DVE-produced operands
(deps on one semaphore merge; distinct semaphores do not).
"""

import numpy as np

import concourse.bass as bass
import concourse.bacc as bacc
import concourse.mybir as mybir
from concourse.tile import TileContext
from concourse.bass_utils import run_bass_kernel_spmd

NUM_USER = 1_000_000
NUM_ITEM = 500_000
F = 64
B = 16384
N_CORES = 8
B_LOC = B // N_CORES  # 2048
P = 128
K = B_LOC // P  # 16
REG = 0.1
LN2 = 0.6931471805599453

_NC_CACHE = {}


def _build_nc(reps=1):
    nc = bacc.Bacc()
    UV = nc.dram_tensor("UV_tab", [NUM_USER + NUM_ITEM, F], mybir.dt.float32,
                        kind="ExternalInput")
    idx_all = nc.dram_tensor("idx_all", [P, 3 * K], mybir.dt.int32,
                             kind="ExternalInput")
    out_o = nc.dram_tensor("out_all", [P, 2 * K + 6], mybir.dt.float32,
                           kind="ExternalOutput")

    AX = mybir.AxisListType
    ACT = mybir.ActivationFunctionType
    f32 = mybir.dt.float32

    with TileContext(nc) as tc:
        with tc.tile_pool(name="pool", bufs=1) as pool:
            idx = pool.tile([P, 3 * K], mybir.dt.int32)
            nc.sync.dma_start(out=idx[:], in_=idx_all[:])
            for _rep in range(reps):
                _emit_body(nc, pool, UV, idx, out_o)
    nc.compile()
    return nc


def _emit_body(nc, pool, UV, idx, out_o):
    AX = mybir.AxisListType
    ACT = mybir.ActivationFunctionType
    f32 = mybir.dt.float32
    if True:
        if True:

            # Gather 6144 rows of 256 B as 48 indirect DMAs of 128 rows each
            # ([P,1] offsets is the HW-proven shape): partition p, block c
            # <- UV[idx[p, c]]. Blocks 0:16 are Uu, 16:32 Vi, 32:48 Vj.
            G = pool.tile([P, 3 * K * F], f32)
            for c in range(3 * K):
                nc.gpsimd.indirect_dma_start(
                    out=G[:, c * F:(c + 1) * F], out_offset=None, in_=UV[:],
                    in_offset=bass.IndirectOffsetOnAxis(ap=idx[:, c:c + 1],
                                                        axis=0))
            Uu = G[:, 0:K * F]
            Vi = G[:, K * F:2 * K * F]
            Vj = G[:, 2 * K * F:3 * K * F]

            t0 = pool.tile([1, 1], f32)
            nc.vector.tensor_copy(out=t0[:], in_=G[:1, 0:1])

            prod_i = pool.tile([P, K * F], f32)
            prod_j = pool.tile([P, K * F], f32)
            y_ui = pool.tile([P, K], f32)
            y_uj = pool.tile([P, K], f32)
            nc.vector.tensor_mul(out=prod_i[:], in0=Uu, in1=Vi)
            nc.vector.reduce_sum(
                out=y_ui[:],
                in_=prod_i[:].rearrange("p (k f) -> p k f", f=F),
                axis=AX.X)
            nc.vector.tensor_mul(out=prod_j[:], in0=Uu, in1=Vj)
            nc.vector.reduce_sum(
                out=y_uj[:],
                in_=prod_j[:].rearrange("p (k f) -> p k f", f=F),
                axis=AX.X)

            # Square-sum of the whole gather on ACT in one op: throwaway main
            # out, accumulate port gives per-partition row sums.
            scr = pool.tile([P, 3 * K * F], f32)
            sq = pool.tile([P, 1], f32)
            nc.scalar.activation(out=scr[:], in_=G[:], func=ACT.Square,
                                 accum_out=sq[:])

            # x = y_ui - y_uj on DVE; |x|, exp, ln chain on ACT.
            x = pool.tile([P, K], f32)
            sum_x = pool.tile([P, 1], f32)
            nc.vector.tensor_sub(out=x[:], in0=y_ui[:], in1=y_uj[:])
            nc.vector.reduce_sum(out=sum_x[:], in_=x[:], axis=AX.X)

            a = pool.tile([P, K], f32)
            e = pool.tile([P, K], f32)
            lg = pool.tile([P, K], f32)
            sum_a = pool.tile([P, 1], f32)
            sum_lg = pool.tile([P, 1], f32)
            nc.scalar.activation(out=a[:], in_=x[:], func=ACT.Abs,
                                 accum_out=sum_a[:])
            nc.scalar.activation(out=e[:], in_=a[:], func=ACT.Exp, scale=-1.0)
            nc.scalar.activation(out=lg[:], in_=e[:], func=ACT.Ln, bias=1.0,
                                 accum_out=sum_lg[:])

            # Stage everything into one DVE-owned tile -> single output DMA
            # (waits on the DVE semaphore only).
            out_t = pool.tile([P, 2 * K + 6], f32)
            nc.vector.tensor_copy(out=out_t[:, 0:K], in_=y_ui[:])
            nc.vector.tensor_copy(out=out_t[:, K:2 * K], in_=y_uj[:])
            nc.vector.tensor_copy(out=out_t[:, 2 * K:2 * K + 1], in_=sum_x[:])
            nc.vector.tensor_copy(out=out_t[:, 2 * K + 1:2 * K + 2], in_=sum_a[:])
            nc.vector.tensor_copy(out=out_t[:, 2 * K + 2:2 * K + 3], in_=sum_lg[:])
            nc.vector.tensor_copy(out=out_t[:, 2 * K + 3:2 * K + 4], in_=sq[:])
            nc.vector.memset(out_t[:, 2 * K + 4:2 * K + 6], 0.0)

            nc.sync.dma_start(out=out_o[:], in_=out_t[:])


def _get_nc(reps=1):
    if reps not in _NC_CACHE:
        _NC_CACHE[reps] = _build_nc(reps)
    return _NC_CACHE[reps]


def _make_in_maps(U, V, u, i, j):
    U = np.asarray(U, dtype=np.float32)
    V = np.asarray(V, dtype=np.float32)
    UV = np.ascontiguousarray(np.concatenate([U, V], axis=0))
    u = np.asarray(u).astype(np.int32).reshape(N_CORES, P, K)
    i = (np.asarray(i).astype(np.int32) + NUM_USER).reshape(N_CORES, P, K)
    j = (np.asarray(j).astype(np.int32) + NUM_USER).reshape(N_CORES, P, K)
    return [
        {
            "UV_tab": UV,
            "idx_all": np.ascontiguousarray(
                np.concatenate([u[c], i[c], j[c]], axis=1)),
        }
        for c in range(N_CORES)
    ]


def _combine(results):
    y_ui = np.concatenate([r["out_all"][:, 0:K].reshape(-1) for r in results])
    y_uj = np.concatenate([r["out_all"][:, K:2 * K].reshape(-1) for r in results])
    sx = sa = sl = sq = 0.0
    for r in results:
        tail = r["out_all"][:, 2 * K:].astype(np.float64)
        sx += tail[:, 0].sum()
        sa += tail[:, 1].sum()
        sl += tail[:, 2].sum()
        sq += tail[:, 3].sum()
    ls = 0.5 * sx - 0.5 * sa - sl
    loss = REG * sq - ls / LN2
    return (
        y_ui.astype(np.float32),
        y_uj.astype(np.float32),
        np.float32(loss),
    )


def kernel(U, V, u, i, j):
    nc = _get_nc()
    in_maps = _make_in_maps(U, V, u, i, j)
    res = run_bass_kernel_spmd(nc, in_maps, list(range(N_CORES)))
    return _combine(res.results)
